# revision 1
# baseline (speedup 1.0000x reference)
"""Multi-head dot-product attention (RoPE, causal) on 8 NeuronCores.

Sharding: data-parallel over batch (2) x tensor-parallel over heads (16 -> 4
per core). Each core projects q/k/v for its 4 heads, runs causal attention,
and computes a partial output projection; the host sums the 4 partials per
batch element.

Device layout notes:
- All matmul operands are float32r (TF32-like, full-rate on the PE for
  moving dims >= 256; every matmul here is emitted at width 512).
- Inputs are fed pre-transposed ([E, S]) so projections contract E on
  partitions; q/k are produced transposed per head ([D, S]).
- Scores are computed transposed (ST[s, t]) so the A@V contraction needs no
  on-chip transposes; softmax skips max-subtraction (scores are O(1) by
  construction). Causality is enforced with 4 precomputed additive mask
  variants (one per diagonal sub-block position) so all matmuls stay full
  width. The 1/rowsum lands after A@V: the denominator is computed by an
  all-ones [128,128] stationary matmul, which leaves the row-sum replicated
  across all partitions, so a single full-lane reciprocal + multiply
  normalizes during PSUM eviction.
- RoPE uses a de-interleaved head dim (even dims | odd dims), folded into a
  host-side permutation of Wq/Wk columns; scores are permutation-invariant.
"""

import numpy as np

B, S, E, N, D = 2, 2048, 2048, 16, 128
HL = 4           # local heads per core (8 cores = 2 batch x 4 head groups)
ND = HL * D      # 512
NT = S // 128    # 16 row tiles
NB = S // 512    # 4 row blocks
NE = E // 128    # 16 contraction tiles
MASK_VALUE = float(-0.7 * np.finfo(np.float32).max)

_NC_CACHE = {}


def _build_module():
    import concourse.bass as bass
    import concourse.mybir as mybir
    import concourse.tile as tile
    from concourse import bacc

    f32 = mybir.dt.float32
    f32r = mybir.dt.float32r
    Exp = mybir.ActivationFunctionType.Exp

    nc = bacc.Bacc("TRN2", target_bir_lowering=False, debug=False, num_devices=8)

    xq_d = nc.dram_tensor("xq_t", [E, S], f32, kind="ExternalInput").ap()
    xkv_d = nc.dram_tensor("xkv_t", [E, S], f32, kind="ExternalInput").ap()
    wq_d = nc.dram_tensor("wq", [E, ND], f32, kind="ExternalInput").ap()
    wk_d = nc.dram_tensor("wk", [E, ND], f32, kind="ExternalInput").ap()
    wv_d = nc.dram_tensor("wv", [E, ND], f32, kind="ExternalInput").ap()
    wo_d = nc.dram_tensor("wo", [ND, E], f32, kind="ExternalInput").ap()
    csd_d = nc.dram_tensor("csd", [128, S], f32, kind="ExternalInput").ap()
    sns_d = nc.dram_tensor("sns", [128, S], f32, kind="ExternalInput").ap()
    ones_d = nc.dram_tensor("ones", [128, 128], f32, kind="ExternalInput").ap()
    msk_d = nc.dram_tensor("msk", [128, 4 * 512], f32, kind="ExternalInput").ap()
    out_d = nc.dram_tensor("out", [S, E], f32, kind="ExternalOutput").ap()

    def load_w_grouped(pool, dram, tag):
        """[E, ND] weights as 4 tiles [128, 4*ND] (4 e-subtiles each)."""
        ws = []
        for eg in range(4):
            w = pool.tile([128, 4 * ND], f32r, tag=f"{tag}{eg}",
                          name=f"{tag}{eg}")
            nc.gpsimd.dma_start(
                w[:].rearrange("p (e n) -> p e n", e=4),
                dram[bass.ds(512 * eg, 512), :].bitcast(f32r)
                .rearrange("(e p) n -> p e n", p=128))
            ws.append(w)
        return ws

    def wslice(ws, et):
        return ws[et // 4][:, bass.ds(512 * (et % 4), 512)]

    with tile.TileContext(nc) as tc:
        with tc.tile_pool(name="qkp", bufs=1) as qk_pool, \
             tc.tile_pool(name="vp", bufs=1) as v_pool:
            qT = [qk_pool.tile([128, S], f32r, tag=f"qT{h}", name=f"qT{h}")
                  for h in range(HL)]
            kT = [qk_pool.tile([128, S], f32r, tag=f"kT{h}", name=f"kT{h}")
                  for h in range(HL)]
            vG = [v_pool.tile([128, 4 * ND], f32r, tag=f"vG{g}",
                              name=f"vG{g}") for g in range(4)]

            # ---- projections (RoPE tables live only here) ----
            with tc.tile_pool(name="tables", bufs=1) as tpool, \
                 tc.tile_pool(name="wkp", bufs=1) as wk_pool:
                csd = tpool.tile([128, S], f32, tag="csd")
                sns = tpool.tile([128, S], f32, tag="sns")

                def rope(dst, src_ps, tb, rope_pool):
                    tbs = bass.ts(tb, 512)
                    tmp = rope_pool.tile([128, 512], f32, tag="tmp",
                                         name="tmp")
                    nc.vector.tensor_mul(tmp[0:64, :], src_ps[64:128, :],
                                         sns[0:64, tbs])
                    nc.vector.tensor_mul(tmp[64:128, :], src_ps[0:64, :],
                                         sns[64:128, tbs])
                    nc.vector.tensor_mul(dst[:, tbs], src_ps[:],
                                         csd[:, tbs])
                    nc.vector.tensor_add(dst[:, tbs], dst[:, tbs], tmp[:])

                # ---- Q projection ----
                with nc.named_scope("proj_q"), \
                     tc.tile_pool(name="wqp", bufs=1) as wq_pool, \
                     tc.tile_pool(name="xq", bufs=3) as xpool, \
                     tc.tile_pool(name="qps", bufs=2, space="PSUM") as qps_pool, \
                     tc.tile_pool(name="rope", bufs=2) as rope_pool:
                    wq = load_w_grouped(wq_pool, wq_d, "wq")
                    nc.gpsimd.dma_start(csd[:], csd_d[:])
                    nc.gpsimd.dma_start(sns[:], sns_d[:])
                    wk = load_w_grouped(wk_pool, wk_d, "wk")
                    for tb in range(NB):
                        qps = [qps_pool.tile([128, 512], f32, tag=f"q{h}",
                                             name=f"qps{h}") for h in range(HL)]
                        for ep in range(NE // 2):  # e-tile pairs
                            x = xpool.tile([128, 2, 512], f32r, tag="x",
                                           name="x")
                            nc.sync.dma_start(
                                x[:],
                                xq_d[bass.ds(256 * ep, 256), bass.ts(tb, 512)]
                                .bitcast(f32r).rearrange("(e p) t -> p e t",
                                                         p=128))
                            for e2 in range(2):
                                et = 2 * ep + e2
                                for h in range(HL):
                                    nc.tensor.matmul(
                                        qps[h][:],
                                        wslice(wq, et)[:, bass.ts(h, 128)],
                                        x[:, e2], start=(et == 0),
                                        stop=(et == NE - 1))
                        for h in range(HL):
                            rope(qT[h], qps[h][:], tb, rope_pool)

                # ---- K + V projection ----
                with nc.named_scope("proj_kv"), \
                     tc.tile_pool(name="wvp", bufs=1) as wv_pool, \
                     tc.tile_pool(name="xkv", bufs=3) as xpool, \
                     tc.tile_pool(name="kps", bufs=1, space="PSUM") as kps_pool, \
                     tc.tile_pool(name="vps", bufs=1, space="PSUM") as vps_pool, \
                     tc.tile_pool(name="rope2", bufs=2) as rope_pool:
                    wv = load_w_grouped(wv_pool, wv_d, "wv")
                    for tb in range(NB):
                        kps = [kps_pool.tile([128, 512], f32, tag=f"k{h}",
                                             name=f"kps{h}") for h in range(HL)]
                        vps = [vps_pool.tile([128, ND], f32, tag=f"v{sv}",
                                             name=f"vps{sv}") for sv in range(4)]
                        for ep in range(NE // 2):
                            x = xpool.tile([128, 2, 512], f32r, tag="x",
                                           name="x")
                            nc.sync.dma_start(
                                x[:],
                                xkv_d[bass.ds(256 * ep, 256), bass.ts(tb, 512)]
                                .bitcast(f32r).rearrange("(e p) t -> p e t",
                                                         p=128))
                            for e2 in range(2):
                                et = 2 * ep + e2
                                for h in range(HL):
                                    nc.tensor.matmul(
                                        kps[h][:],
                                        wslice(wk, et)[:, bass.ts(h, 128)],
                                        x[:, e2], start=(et == 0),
                                        stop=(et == NE - 1))
                                for sv in range(4):
                                    nc.tensor.matmul(
                                        vps[sv][:], x[:, e2, bass.ts(sv, 128)],
                                        wslice(wv, et), start=(et == 0),
                                        stop=(et == NE - 1))
                        for h in range(HL):
                            rope(kT[h], kps[h][:], tb, rope_pool)
                        for sv in range(4):
                            nc.scalar.copy(vG[tb][:, bass.ts(sv, 512)],
                                           vps[sv][:])

            # ---- Attention ----
            with tc.tile_pool(name="uTp", bufs=1) as ut_pool:
                uT = [ut_pool.tile([128, S], f32r, tag=f"uT{h}", name=f"uT{h}")
                      for h in range(HL)]
                with nc.named_scope("attn"), \
                     tc.tile_pool(name="cst", bufs=1) as cpool, \
                     tc.tile_pool(name="et", bufs=1) as et_pool, \
                     tc.tile_pool(name="sps", bufs=2, space="PSUM") as sps_pool, \
                     tc.tile_pool(name="dps", bufs=2, space="PSUM") as dps_pool, \
                     tc.tile_pool(name="ups", bufs=2, space="PSUM") as ups_pool, \
                     tc.tile_pool(name="rcp", bufs=2) as rcp_pool:
                    ones = cpool.tile([128, 128], f32r, tag="ones")
                    msk = cpool.tile([128, 4 * 512], f32, tag="msk")
                    nc.gpsimd.dma_start(ones[:], ones_d[:].bitcast(f32r))
                    nc.gpsimd.dma_start(msk[:], msk_d[:])
                    eG = [et_pool.tile([128, 2048], f32r, tag=f"eG{g}",
                                       name=f"eG{g}") for g in range(4)]

                    def e_ap(si, w=512):
                        base = 512 * (si % 4)
                        return eG[si // 4][:, base:base + w]

                    for h in range(HL):
                        for tb in range(NB):
                            nsi = 4 * (tb + 1)
                            tbs = bass.ts(tb, 512)
                            for j in range(nsi // 2):  # s-tile pairs
                                sp = sps_pool.tile([128, 2, 512], f32,
                                                   tag="sp", name="sp")
                                for p2 in range(2):
                                    si = 2 * j + p2
                                    nc.tensor.matmul(
                                        sp[:, p2], kT[h][:, bass.ts(si, 128)],
                                        qT[h][:, tbs], start=True, stop=True)
                                    v = si - 4 * tb
                                    if v >= 0:
                                        nc.vector.tensor_add(
                                            sp[:, p2], sp[:, p2],
                                            msk[:, bass.ts(v, 512)])
                                nc.scalar.activation(
                                    eG[j // 2][:, bass.ts(j % 2, 1024)],
                                    sp[:].rearrange("p a b -> p (a b)"), Exp)
                            den = dps_pool.tile([128, 512], f32, tag="den",
                                                name="den")
                            for si in range(nsi):
                                nc.tensor.matmul(den[:], ones[:], e_ap(si),
                                                 start=(si == 0),
                                                 stop=(si == nsi - 1))
                            rec = rcp_pool.tile([128, 512], f32, tag="rec",
                                                name="rec")
                            nc.vector.reciprocal(rec[:], den[:])
                            up = ups_pool.tile([128, 512], f32, tag="up",
                                               name="up")
                            for si in range(nsi):
                                g, sv = si // 4, si % 4
                                nc.tensor.matmul(
                                    up[:],
                                    vG[g][:, 512 * sv + 128 * h:
                                          512 * sv + 128 * (h + 1)],
                                    e_ap(si), start=(si == 0),
                                    stop=(si == nsi - 1))
                            nc.vector.tensor_mul(uT[h][:, tbs], up[:], rec[:])

                # ---- Output projection ----
                with nc.named_scope("out_proj"), \
                     tc.tile_pool(name="wop", bufs=1) as wo_pool, \
                     tc.tile_pool(name="ops", bufs=2, space="PSUM") as ops_pool, \
                     tc.tile_pool(name="ob", bufs=3) as ob_pool:
                    wo = []
                    for h in range(HL):
                        w = wo_pool.tile([128, E], f32r, tag=f"wo{h}",
                                         name=f"wo{h}")
                        nc.gpsimd.dma_start(
                            w[:], wo_d[bass.ts(h, 128), :].bitcast(f32r))
                        wo.append(w)
                    for tt in range(NT):
                        op = ops_pool.tile([128, E], f32, tag="op", name="op")
                        for h in range(HL):
                            for ec in range(4):
                                nc.tensor.matmul(
                                    op[:, bass.ts(ec, 512)],
                                    uT[h][:, bass.ts(tt, 128)],
                                    wo[h][:, bass.ts(ec, 512)],
                                    start=(h == 0), stop=(h == HL - 1))
                        ob = ob_pool.tile([128, E], f32, tag="ob", name="ob")
                        nc.scalar.copy(ob[:], op[:])
                        nc.sync.dma_start(out_d[bass.ts(tt, 128), :], ob[:])

    nc.compile()
    return nc


def _get_module():
    if "nc" not in _NC_CACHE:
        _NC_CACHE["nc"] = _build_module()
    return _NC_CACHE["nc"]


def _host_prep(inputs_q, inputs_kv, positions, Wq, Wk, Wv, Wo):
    """Build the 8 per-core input maps."""
    perm = np.concatenate([np.arange(0, D, 2), np.arange(1, D, 2)])  # de-interleave
    scale = np.float32(1.0 / np.sqrt(D))
    half = D // 2
    timescale = 10000.0 ** (2.0 * np.arange(half, dtype=np.float64) / D)
    ones = np.ones((128, 128), dtype=np.float32)
    # mask variant v (diag sub-block at cols [128v, 128v+128)):
    # masked (additive MASK_VALUE) where col < 128*v + row
    s_i = np.arange(128)[:, None]
    c_i = np.arange(512)[None, :]
    msk = np.concatenate(
        [np.where(c_i < 128 * v + s_i, MASK_VALUE, 0.0) for v in range(4)],
        axis=1).astype(np.float32)

    in_maps = []
    for c in range(8):
        b = c // 4
        h0 = (c % 4) * HL
        angle = positions[b].astype(np.float64)[None, :] / timescale[:, None]  # [64,S]
        cs = np.cos(angle).astype(np.float32)
        sn = np.sin(angle).astype(np.float32)
        csd = np.concatenate([cs, cs], axis=0)               # [128, S]
        sns = np.concatenate([-sn, sn], axis=0)              # [128, S]
        wq = (Wq[:, h0:h0 + HL, :][:, :, perm] * scale).reshape(E, ND)
        wk = Wk[:, h0:h0 + HL, :][:, :, perm].reshape(E, ND)
        wv = Wv[:, h0:h0 + HL, :].reshape(E, ND)
        wo = Wo[h0:h0 + HL].reshape(ND, E)
        in_maps.append({
            "xq_t": np.ascontiguousarray(inputs_q[b].T),
            "xkv_t": np.ascontiguousarray(inputs_kv[b].T),
            "wq": np.ascontiguousarray(wq.astype(np.float32)),
            "wk": np.ascontiguousarray(wk.astype(np.float32)),
            "wv": np.ascontiguousarray(wv.astype(np.float32)),
            "wo": np.ascontiguousarray(wo.astype(np.float32)),
            "csd": csd, "sns": sns, "ones": ones, "msk": msk,
        })
    return in_maps


def kernel(inputs_q, inputs_kv, positions, Wq, Wk, Wv, Wo, _trace=False,
           _trace_kwargs=None):
    from concourse import bass_utils

    nc = _get_module()
    in_maps = _host_prep(inputs_q, inputs_kv, positions, Wq, Wk, Wv, Wo)
    res = bass_utils.run_bass_kernel_spmd(
        nc, in_maps, core_ids=list(range(8)), trace=_trace,
        **(_trace_kwargs or {}))
    if _trace:
        _NC_CACHE["last_results"] = res
    parts = [res.results[c]["out"] for c in range(8)]
    out0 = parts[0] + parts[1] + parts[2] + parts[3]
    out1 = parts[4] + parts[5] + parts[6] + parts[7]
    return np.stack([out0, out1]).astype(np.float32)



# revision 4
# speedup vs baseline: 1.1938x; 1.1938x over previous
"""Multi-head dot-product attention (RoPE, causal) on 8 NeuronCores.

Sharding: data-parallel over batch (2) x tensor-parallel over heads (16 -> 4
per core). Each core projects q/k/v for its 4 heads, runs causal attention,
and computes a partial output projection; the host sums the 4 partials per
batch element.

v2 design notes (vs the f32r baseline):
- All matmul operands are bf16 (PSUM accumulation stays f32): same PE
  throughput as f32r but half the DMA bytes and SBUF footprint. Host packs
  every DRAM tensor in the exact SBUF layout so all loads are full-line
  contiguous copies.
- Phase order: KV projection (all 4 t-blocks) -> per t-block [Q projection +
  attention of the previous t-block interleaved] -> output projection. The
  PE instruction stream never waits on a phase boundary: attention t-block
  tb only needs KV blocks <= tb and the Q block produced just before it.
- Attention keeps the transposed-scores layout: ST[s, t] so A@V needs no
  transposes, additive causal masks on the 4 diagonal sub-blocks only
  (width-trimmed), softmax denominator via an all-ones stationary matmul,
  reciprocal via Ln/Exp on the scalar engine (same activation table set as
  the softmax Exp), normalization during PSUM eviction.
- Cross-head interleave: the PE stream runs QK(h+1) between QK(h) and
  den/AV(h) so the scalar engine's exp latency is hidden; two eG buffers
  (even/odd head) break the WAR chain between consecutive heads.
- PSUM: q-projection accumulators share the attention score pool slots
  (3 x [128,2,512] = 6 banks) + den (1) + AV (1) = 8 banks exactly.
"""

import numpy as np

B, S, E, N, D = 2, 2048, 2048, 16, 128
HL = 4           # local heads per core (8 cores = 2 batch x 4 head groups)
ND = HL * D      # 512
NT = S // 128    # 16 row tiles
NB = S // 512    # 4 row blocks
NE = E // 128    # 16 contraction tiles
MASK_VALUE = float(-0.7 * np.finfo(np.float32).max)
MW = [128, 256, 384, 512]            # mask widths per diagonal variant
MOFF = [0, 128, 384, 768]            # col offsets of variants in msk table

_NC_CACHE = {}


def _build_module():
    import concourse.bass as bass
    import concourse.mybir as mybir
    import concourse.tile as tile
    from concourse import bacc

    f32 = mybir.dt.float32
    bf16 = mybir.dt.bfloat16
    Exp = mybir.ActivationFunctionType.Exp
    Ln = mybir.ActivationFunctionType.Ln

    nc = bacc.Bacc("TRN2", target_bir_lowering=False, debug=False, num_devices=8)

    # Host-packed layouts (flat contiguous DMAs):
    xq_d = nc.dram_tensor("xq_p", [NB, 128, NE, 512], bf16, kind="ExternalInput").ap()
    xkv_d = nc.dram_tensor("xkv_p", [NB, 128, NE, 512], bf16, kind="ExternalInput").ap()
    wq_d = nc.dram_tensor("wq_p", [128, NE, ND], bf16, kind="ExternalInput").ap()
    wk_d = nc.dram_tensor("wk_p", [128, NE, ND], bf16, kind="ExternalInput").ap()
    wv_d = nc.dram_tensor("wv_p", [128, NE, ND], bf16, kind="ExternalInput").ap()
    wo_d = nc.dram_tensor("wo_p", [128, HL, E], bf16, kind="ExternalInput").ap()
    csd_d = nc.dram_tensor("csd", [128, S], f32, kind="ExternalInput").ap()
    sns_d = nc.dram_tensor("sns", [128, S], f32, kind="ExternalInput").ap()
    ones_d = nc.dram_tensor("ones", [128, 128], bf16, kind="ExternalInput").ap()
    msk_d = nc.dram_tensor("msk", [128, 1280], f32, kind="ExternalInput").ap()
    out_d = nc.dram_tensor("out", [NT, 128, E], f32, kind="ExternalOutput").ap()

    with tile.TileContext(nc) as tc:
        with tc.tile_pool(name="const", bufs=1) as cpool, \
             tc.tile_pool(name="wqo", bufs=1) as wqo_pool, \
             tc.tile_pool(name="persist", bufs=1) as pers_pool:
            csd = cpool.tile([128, S], f32, tag="csd")
            sns = cpool.tile([128, S], f32, tag="sns")
            msk = cpool.tile([128, 1280], f32, tag="msk")
            ones = cpool.tile([128, 128], bf16, tag="ones")
            wq = wqo_pool.tile([128, NE, ND], bf16, tag="wq")
            wo = wqo_pool.tile([128, HL, E], bf16, tag="wo")
            kT = [pers_pool.tile([128, S], bf16, tag=f"kT{h}", name=f"kT{h}")
                  for h in range(HL)]
            vG = [pers_pool.tile([128, 4, ND], bf16, tag=f"vG{g}", name=f"vG{g}")
                  for g in range(NB)]
            uT = [pers_pool.tile([128, S], bf16, tag=f"uT{h}", name=f"uT{h}")
                  for h in range(HL)]

            def rope(dst, src, tb, pool):
                """dst[:, tb-block] = rope(src) with de-interleaved head dim.
                src is a [128, 512] f32 PSUM AP; dst is bf16 SBUF."""
                tbs = bass.ts(tb, 512)
                tmp = pool.tile([128, 512], f32, tag="tmp", name="tmp")
                tmp2 = pool.tile([128, 512], f32, tag="tmp2", name="tmp2")
                nc.vector.tensor_mul(tmp[0:64, :], src[64:128, :], sns[0:64, tbs])
                nc.vector.tensor_mul(tmp[64:128, :], src[0:64, :], sns[64:128, tbs])
                nc.vector.tensor_mul(tmp2[:], src[:], csd[:, tbs])
                nc.vector.tensor_add(dst[:, tbs], tmp2[:], tmp[:])

            # ---------------- Phase 1: K + V projection ----------------
            with nc.named_scope("proj_kv"), \
                 tc.tile_pool(name="wkv", bufs=1) as wkv_pool, \
                 tc.tile_pool(name="xkv", bufs=2) as xkv_pool, \
                 tc.tile_pool(name="kvps", bufs=2, space="PSUM") as kvps_pool, \
                 tc.tile_pool(name="rope_kv", bufs=2) as rkv_pool:
                wk = wkv_pool.tile([128, NE, ND], bf16, tag="wk")
                wv = wkv_pool.tile([128, NE, ND], bf16, tag="wv")
                # Preloads. gpsimd queue order = priority order.
                nc.gpsimd.dma_start(wk[:], wk_d[:])
                nc.gpsimd.dma_start(wv[:], wv_d[:])
                nc.gpsimd.dma_start(csd[:], csd_d[:])
                nc.gpsimd.dma_start(sns[:], sns_d[:])
                nc.gpsimd.dma_start(msk[:], msk_d[:])
                nc.gpsimd.dma_start(ones[:], ones_d[:])
                nc.gpsimd.dma_start(wq[:], wq_d[:])
                nc.gpsimd.dma_start(wo[:], wo_d[:])

                for tb in range(NB):
                    xk = xkv_pool.tile([128, NE, 512], bf16, tag="xk",
                                       name=f"xk{tb}")
                    nc.sync.dma_start(xk[:], xkv_d[tb])
                    for pp in range(2):   # 2 half-passes: 2 heads + 2 s-subtiles
                        kps = kvps_pool.tile([128, 2, 512], f32, tag="kps",
                                             name=f"kps{tb}{pp}")
                        vps = kvps_pool.tile([128, 2, 512], f32, tag="vps",
                                             name=f"vps{tb}{pp}")
                        for et in range(NE):
                            for i in range(2):
                                h = 2 * pp + i
                                nc.tensor.matmul(
                                    kps[:, i], wk[:, et, bass.ts(h, 128)],
                                    xk[:, et, :], start=(et == 0),
                                    stop=(et == NE - 1))
                            for i in range(2):
                                sv = 2 * pp + i
                                nc.tensor.matmul(
                                    vps[:, i], xk[:, et, bass.ts(sv, 128)],
                                    wv[:, et, :], start=(et == 0),
                                    stop=(et == NE - 1))
                        for i in range(2):
                            rope(kT[2 * pp + i], kps[:, i], tb, rkv_pool)
                            nc.scalar.copy(vG[tb][:, 2 * pp + i, :], vps[:, i])

            # ---------- Phase 2+3: Q projection + attention, interleaved ----------
            with nc.named_scope("q_attn"), \
                 tc.tile_pool(name="qat", bufs=1) as qat_pool, \
                 tc.tile_pool(name="xq", bufs=2) as xq_pool, \
                 tc.tile_pool(name="sps", bufs=3, space="PSUM") as sps_pool, \
                 tc.tile_pool(name="dps", bufs=1, space="PSUM") as dps_pool, \
                 tc.tile_pool(name="ups", bufs=1, space="PSUM") as ups_pool, \
                 tc.tile_pool(name="rope_q", bufs=2) as rq_pool, \
                 tc.tile_pool(name="rcp", bufs=2) as rcp_pool:
                qT = [qat_pool.tile([128, S], bf16, tag=f"qT{h}", name=f"qT{h}")
                      for h in range(HL)]
                # two eG sets: even heads use set 0, odd heads set 1
                eG = [[qat_pool.tile([128, 2048], bf16, tag=f"eG{p}{g}",
                                     name=f"eG{p}{g}") for g in range(4)]
                      for p in range(2)]

                def e_ap(eset, si):
                    return eset[si // 4][:, bass.ds(512 * (si % 4), 512)]

                xq_tiles = {}

                def q_mm(tb, hp):
                    """Project heads (2hp, 2hp+1) for t-block tb. Returns psum."""
                    qps = sps_pool.tile([128, 2, 512], f32, tag="sp",
                                        name=f"qps{tb}{hp}")
                    xqt = xq_tiles[tb]
                    for et in range(NE):
                        for i in range(2):
                            h = 2 * hp + i
                            nc.tensor.matmul(
                                qps[:, i], wq[:, et, bass.ts(h, 128)],
                                xqt[:, et, :], start=(et == 0),
                                stop=(et == NE - 1))
                    return qps

                def q_rope(tb, hp, qps):
                    for i in range(2):
                        rope(qT[2 * hp + i], qps[:, i], tb, rq_pool)

                def attn_qk(tb, h):
                    """Scores + exp for head h of t-block tb."""
                    nsi = 4 * (tb + 1)
                    eset = eG[h % 2]
                    for j in range(nsi // 2):
                        sp = sps_pool.tile([128, 2, 512], f32, tag="sp",
                                           name=f"sp{tb}{h}{j}")
                        for p2 in range(2):
                            si = 2 * j + p2
                            nc.tensor.matmul(
                                sp[:, p2], kT[h][:, bass.ts(si, 128)],
                                qT[h][:, bass.ts(tb, 512)], start=True,
                                stop=True)
                            v = si - 4 * tb
                            if v >= 0:
                                w = MW[v]
                                nc.vector.tensor_add(
                                    sp[:, p2, 0:w], sp[:, p2, 0:w],
                                    msk[:, bass.ds(MOFF[v], w)])
                        nc.scalar.activation(
                            eset[j // 2][:, bass.ts(j % 2, 1024)],
                            sp[:].rearrange("p a b -> p (a b)"), Exp)

                def attn_dv(tb, h):
                    """Denominator + A@V + normalization for head h."""
                    nsi = 4 * (tb + 1)
                    eset = eG[h % 2]
                    den = dps_pool.tile([128, 512], f32, tag="den",
                                        name=f"den{tb}{h}")
                    for si in range(nsi):
                        nc.tensor.matmul(den[:], ones[:], e_ap(eset, si),
                                         start=(si == 0), stop=(si == nsi - 1))
                    lnt = rcp_pool.tile([128, 512], f32, tag="lnt", name="lnt")
                    rec = rcp_pool.tile([128, 512], f32, tag="rec", name="rec")
                    nc.scalar.activation(lnt[:], den[:], Ln)
                    nc.scalar.activation(rec[:], lnt[:], Exp, scale=-1.0)
                    up = ups_pool.tile([128, 512], f32, tag="up",
                                       name=f"up{tb}{h}")
                    for si in range(nsi):
                        g, sv = si // 4, si % 4
                        nc.tensor.matmul(up[:], vG[g][:, sv, bass.ts(h, 128)],
                                         e_ap(eset, si), start=(si == 0),
                                         stop=(si == nsi - 1))
                    nc.vector.tensor_mul(uT[h][:, bass.ts(tb, 512)], up[:],
                                         rec[:])

                def attn_block(tb, ropes=None):
                    """Full attention t-block with cross-head PE interleave."""
                    attn_qk(tb, 0)
                    attn_qk(tb, 1)
                    attn_dv(tb, 0)
                    if ropes is not None:
                        ropes()
                    attn_qk(tb, 2)
                    attn_dv(tb, 1)
                    attn_qk(tb, 3)
                    attn_dv(tb, 2)
                    attn_dv(tb, 3)

                def load_xq(tb):
                    xqt = xq_pool.tile([128, NE, 512], bf16, tag="xq",
                                       name=f"xq{tb}")
                    nc.sync.dma_start(xqt[:], xq_d[tb])
                    xq_tiles[tb] = xqt

                load_xq(0)
                load_xq(1)
                qps = q_mm(0, 0)
                q_rope(0, 0, qps)
                qps = q_mm(0, 1)
                q_rope(0, 1, qps)
                for tb in range(1, NB):
                    if tb + 1 < NB:
                        load_xq(tb + 1)
                    qps0 = q_mm(tb, 0)
                    attn_block(tb - 1, ropes=lambda: q_rope(tb, 0, qps0))
                    qps1 = q_mm(tb, 1)
                    q_rope(tb, 1, qps1)
                attn_block(NB - 1)

            # ---------------- Phase 4: output projection ----------------
            with nc.named_scope("out_proj"), \
                 tc.tile_pool(name="ops", bufs=2, space="PSUM") as ops_pool, \
                 tc.tile_pool(name="ob", bufs=3) as ob_pool:
                for tt in range(NT):
                    op = ops_pool.tile([128, E], f32, tag="op", name=f"op{tt}")
                    for ec in range(4):
                        for h in range(HL):
                            nc.tensor.matmul(
                                op[:, bass.ts(ec, 512)],
                                uT[h][:, bass.ts(tt, 128)],
                                wo[:, h, bass.ts(ec, 512)],
                                start=(h == 0), stop=(h == HL - 1))
                    ob = ob_pool.tile([128, E], f32, tag="ob", name=f"ob{tt}")
                    nc.scalar.copy(ob[:], op[:])
                    nc.sync.dma_start(out_d[tt], ob[:])

    nc.compile()
    return nc


def _get_module():
    if "nc" not in _NC_CACHE:
        _NC_CACHE["nc"] = _build_module()
    return _NC_CACHE["nc"]


def _host_prep(inputs_q, inputs_kv, positions, Wq, Wk, Wv, Wo):
    """Build the 8 per-core input maps (device-packed layouts, bf16)."""
    import ml_dtypes
    bf16 = ml_dtypes.bfloat16
    perm = np.concatenate([np.arange(0, D, 2), np.arange(1, D, 2)])  # de-interleave
    scale = np.float32(1.0 / np.sqrt(D))
    half = D // 2
    timescale = 10000.0 ** (2.0 * np.arange(half, dtype=np.float64) / D)
    ones = np.ones((128, 128), dtype=bf16)
    # mask variant v (diag sub-block at cols [128v, 128v+128)), width-trimmed:
    # masked (additive MASK_VALUE) where col < 128*v + row
    s_i = np.arange(128)[:, None]
    msk = np.concatenate(
        [np.where(np.arange(MW[v])[None, :] < 128 * v + s_i, MASK_VALUE, 0.0)
         for v in range(4)], axis=1).astype(np.float32)

    def pack_x(xT):
        # [E, S] f32 -> [NB, 128, NE, 512]: x_p[tb, p, et, t] = xT[128 et + p, 512 tb + t]
        return np.ascontiguousarray(
            xT.reshape(NE, 128, NB, 512).transpose(2, 1, 0, 3).astype(bf16))

    def pack_w(w):
        # [E, ND] -> [128, NE, ND]: w_p[p, et, n] = w[128 et + p, n]
        return np.ascontiguousarray(
            w.reshape(NE, 128, ND).transpose(1, 0, 2).astype(bf16))

    in_maps = []
    for c in range(8):
        b = c // 4
        h0 = (c % 4) * HL
        angle = positions[b].astype(np.float64)[None, :] / timescale[:, None]  # [64,S]
        cs = np.cos(angle).astype(np.float32)
        sn = np.sin(angle).astype(np.float32)
        csd = np.concatenate([cs, cs], axis=0)               # [128, S]
        sns = np.concatenate([-sn, sn], axis=0)              # [128, S]
        wq = (Wq[:, h0:h0 + HL, :][:, :, perm] * scale).reshape(E, ND)
        wk = Wk[:, h0:h0 + HL, :][:, :, perm].reshape(E, ND)
        wv = Wv[:, h0:h0 + HL, :].reshape(E, ND)
        wo = Wo[h0:h0 + HL]                                   # [HL, D, E]
        in_maps.append({
            "xq_p": pack_x(np.asarray(inputs_q[b]).T),
            "xkv_p": pack_x(np.asarray(inputs_kv[b]).T),
            "wq_p": pack_w(np.asarray(wq, dtype=np.float32)),
            "wk_p": pack_w(np.asarray(wk, dtype=np.float32)),
            "wv_p": pack_w(np.asarray(wv, dtype=np.float32)),
            "wo_p": np.ascontiguousarray(
                np.asarray(wo, dtype=np.float32).transpose(1, 0, 2).astype(bf16)),
            "csd": csd, "sns": sns, "ones": ones, "msk": msk,
        })
    return in_maps


def kernel(inputs_q, inputs_kv, positions, Wq, Wk, Wv, Wo, _trace=False,
           _trace_kwargs=None):
    from concourse import bass_utils

    nc = _get_module()
    in_maps = _host_prep(inputs_q, inputs_kv, positions, Wq, Wk, Wv, Wo)
    res = bass_utils.run_bass_kernel_spmd(
        nc, in_maps, core_ids=list(range(8)), trace=_trace,
        **(_trace_kwargs or {}))
    if _trace:
        _NC_CACHE["last_results"] = res
    parts = [np.asarray(res.results[c]["out"], dtype=np.float32).reshape(S, E)
             for c in range(8)]
    out0 = parts[0] + parts[1] + parts[2] + parts[3]
    out1 = parts[4] + parts[5] + parts[6] + parts[7]
    return np.stack([out0, out1]).astype(np.float32)


# revision 15
# speedup vs baseline: 1.2113x; 1.0147x over previous
"""Multi-head dot-product attention (RoPE, causal) on 8 NeuronCores.

Sharding: data-parallel over batch (2) x tensor-parallel over heads (16 -> 4
per core). Each core projects q/k/v for its 4 heads, runs causal attention,
and computes a partial output projection; the host sums the 4 partials per
batch element.

v2 design notes (vs the f32r baseline):
- All matmul operands are bf16 (PSUM accumulation stays f32): same PE
  throughput as f32r but half the DMA bytes and SBUF footprint. Host packs
  every DRAM tensor in the exact SBUF layout so all loads are full-line
  contiguous copies.
- Phase order: KV projection (all 4 t-blocks) -> per t-block [Q projection +
  attention of the previous t-block interleaved] -> output projection. The
  PE instruction stream never waits on a phase boundary: attention t-block
  tb only needs KV blocks <= tb and the Q block produced just before it.
- Attention keeps the transposed-scores layout: ST[s, t] so A@V needs no
  transposes, additive causal masks on the 4 diagonal sub-blocks only
  (width-trimmed), softmax denominator via an all-ones stationary matmul,
  reciprocal via Ln/Exp on the scalar engine (same activation table set as
  the softmax Exp), normalization during PSUM eviction.
- Cross-head interleave: the PE stream runs QK(h+1) between QK(h) and
  den/AV(h) so the scalar engine's exp latency is hidden; two eG buffers
  (even/odd head) break the WAR chain between consecutive heads.
- PSUM: q-projection accumulators share the attention score pool slots
  (3 x [128,2,512] = 6 banks) + den (1) + AV (1) = 8 banks exactly.
"""

import numpy as np

B, S, E, N, D = 2, 2048, 2048, 16, 128
HL = 4           # local heads per core (8 cores = 2 batch x 4 head groups)
ND = HL * D      # 512
NT = S // 128    # 16 row tiles
NB = S // 512    # 4 row blocks
NE = E // 128    # 16 contraction tiles
MASK_VALUE = float(-0.7 * np.finfo(np.float32).max)
MW = [128, 256, 384, 512]            # mask widths per diagonal variant
MOFF = [0, 128, 384, 768]            # col offsets of variants in msk table

_NC_CACHE = {}


def _build_module():
    import concourse.bass as bass
    import concourse.mybir as mybir
    import concourse.tile as tile
    from concourse import bacc

    f32 = mybir.dt.float32
    bf16 = mybir.dt.bfloat16
    Exp = mybir.ActivationFunctionType.Exp

    nc = bacc.Bacc("TRN2", target_bir_lowering=False, debug=False, num_devices=8)

    # Host-packed layouts (flat contiguous DMAs):
    xq_d = nc.dram_tensor("xq_p", [NB, 128, NE, 512], bf16, kind="ExternalInput").ap()
    xkv_d = nc.dram_tensor("xkv_p", [NB, 128, NE, 512], bf16, kind="ExternalInput").ap()
    wq_d = nc.dram_tensor("wq_p", [128, NE, ND], bf16, kind="ExternalInput").ap()
    wk_d = nc.dram_tensor("wk_p", [128, NE, ND], bf16, kind="ExternalInput").ap()
    wv_d = nc.dram_tensor("wv_p", [128, NE, ND], bf16, kind="ExternalInput").ap()
    wo_d = nc.dram_tensor("wo_p", [128, HL, E], bf16, kind="ExternalInput").ap()
    csd_d = nc.dram_tensor("csd", [128, S], f32, kind="ExternalInput").ap()
    sns_d = nc.dram_tensor("sns", [128, S], f32, kind="ExternalInput").ap()
    ones_d = nc.dram_tensor("ones", [128, 128], bf16, kind="ExternalInput").ap()
    msk_d = nc.dram_tensor("msk", [128, 1280], f32, kind="ExternalInput").ap()
    out_d = nc.dram_tensor("out", [NT, 128, E], bf16, kind="ExternalOutput").ap()

    with tile.TileContext(nc) as tc:
        with tc.tile_pool(name="const", bufs=1) as cpool, \
             tc.tile_pool(name="wqo", bufs=1) as wqo_pool, \
             tc.tile_pool(name="xq", bufs=2) as xq_pool, \
             tc.tile_pool(name="persist", bufs=1) as pers_pool:
            csd = cpool.tile([128, S], f32, tag="csd")
            sns = cpool.tile([128, S], f32, tag="sns")
            msk = cpool.tile([128, 1280], f32, tag="msk")
            ones = cpool.tile([128, 128], bf16, tag="ones")
            wq = wqo_pool.tile([128, NE, ND], bf16, tag="wq")
            wo = wqo_pool.tile([128, HL, E], bf16, tag="wo")
            kT = [pers_pool.tile([128, S], bf16, tag=f"kT{h}", name=f"kT{h}")
                  for h in range(HL)]
            vG = [pers_pool.tile([128, 4, ND], bf16, tag=f"vG{g}", name=f"vG{g}")
                  for g in range(NB)]
            uT = [pers_pool.tile([128, S], bf16, tag=f"uT{h}", name=f"uT{h}")
                  for h in range(HL)]
            xq_tiles = {}

            def load_xq(tb):
                xqt = xq_pool.tile([128, NE, 512], bf16, tag="xq",
                                   name=f"xq{tb}")
                nc.sync.dma_start(xqt[:], xq_d[tb])
                xq_tiles[tb] = xqt

            def rope(dst, src, tb, pool):
                """dst[:, tb-block] = rope(src) with de-interleaved head dim.
                src is a [128, 512] f32 PSUM AP; dst is bf16 SBUF."""
                tbs = bass.ts(tb, 512)
                tmp = pool.tile([128, 512], f32, tag="tmp", name="tmp")
                tmp2 = pool.tile([128, 512], f32, tag="tmp2", name="tmp2")
                nc.vector.tensor_mul(tmp[0:64, :], src[64:128, :], sns[0:64, tbs])
                nc.vector.tensor_mul(tmp[64:128, :], src[0:64, :], sns[64:128, tbs])
                nc.vector.tensor_mul(tmp2[:], src[:], csd[:, tbs])
                nc.vector.tensor_add(dst[:, tbs], tmp2[:], tmp[:])

            # ---------------- Phase 1: K + V projection ----------------
            with nc.named_scope("proj_kv"), \
                 tc.tile_pool(name="wkv", bufs=1) as wkv_pool, \
                 tc.tile_pool(name="xkv", bufs=2) as xkv_pool, \
                 tc.tile_pool(name="kvps", bufs=2, space="PSUM") as kvps_pool, \
                 tc.tile_pool(name="rope_kv", bufs=2) as rkv_pool:
                wk = wkv_pool.tile([128, NE, ND], bf16, tag="wk")
                wv = wkv_pool.tile([128, NE, ND], bf16, tag="wv")
                # Preloads. Queue order = need order; wk + the first xkv
                # block are chunked so the first matmul chain starts early.
                for ch in range(4):
                    nc.gpsimd.dma_start(wk[:, 4 * ch:4 * (ch + 1), :],
                                        wk_d[:, 4 * ch:4 * (ch + 1), :])
                nc.gpsimd.dma_start(wv[:], wv_d[:])
                nc.gpsimd.dma_start(csd[:], csd_d[:])
                nc.gpsimd.dma_start(sns[:], sns_d[:])
                nc.gpsimd.dma_start(msk[:], msk_d[:])
                nc.gpsimd.dma_start(ones[:], ones_d[:])
                nc.gpsimd.dma_start(wq[:], wq_d[:])
                nc.gpsimd.dma_start(wo[:], wo_d[:])

                for tb in range(NB):
                    xk = xkv_pool.tile([128, NE, 512], bf16, tag="xk",
                                       name=f"xk{tb}")
                    if tb == 0:
                        for ch in range(4):
                            nc.sync.dma_start(
                                xk[:, 4 * ch:4 * (ch + 1), :],
                                xkv_d[tb][:, 4 * ch:4 * (ch + 1), :])
                    else:
                        nc.sync.dma_start(xk[:], xkv_d[tb])
                    if tb == 1:
                        # Dispatch the first two Q blocks between xkv[1] and
                        # xkv[2] on the sync queue so Q projection never waits.
                        load_xq(0)
                        load_xq(1)
                    for pp in range(2):   # 2 half-passes: 2 heads + 2 s-subtiles
                        kps = kvps_pool.tile([128, 2, 512], f32, tag="kps",
                                             name=f"kps{tb}{pp}")
                        vps = kvps_pool.tile([128, 2, 512], f32, tag="vps",
                                             name=f"vps{tb}{pp}")
                        for et in range(NE):
                            for i in range(2):
                                h = 2 * pp + i
                                nc.tensor.matmul(
                                    kps[:, i], wk[:, et, bass.ts(h, 128)],
                                    xk[:, et, :], start=(et == 0),
                                    stop=(et == NE - 1))
                            for i in range(2):
                                sv = 2 * pp + i
                                nc.tensor.matmul(
                                    vps[:, i], xk[:, et, bass.ts(sv, 128)],
                                    wv[:, et, :], start=(et == 0),
                                    stop=(et == NE - 1))
                        for i in range(2):
                            rope(kT[2 * pp + i], kps[:, i], tb, rkv_pool)
                            nc.scalar.copy(vG[tb][:, 2 * pp + i, :], vps[:, i])

            # ---------- Phase 2+3: Q projection + attention, interleaved ----------
            with nc.named_scope("q_attn"), \
                 tc.tile_pool(name="qat", bufs=1) as qat_pool, \
                 tc.tile_pool(name="sps", bufs=3, space="PSUM") as sps_pool, \
                 tc.tile_pool(name="dps", bufs=1, space="PSUM") as dps_pool, \
                 tc.tile_pool(name="ups", bufs=1, space="PSUM") as ups_pool, \
                 tc.tile_pool(name="rope_q", bufs=2) as rq_pool, \
                 tc.tile_pool(name="rcp", bufs=2) as rcp_pool:
                qT = [qat_pool.tile([128, S], bf16, tag=f"qT{h}", name=f"qT{h}")
                      for h in range(HL)]
                # two eG sets: even heads use set 0, odd heads set 1
                eG = [[qat_pool.tile([128, 2048], bf16, tag=f"eG{p}{g}",
                                     name=f"eG{p}{g}") for g in range(4)]
                      for p in range(2)]

                def e_ap(eset, si):
                    return eset[si // 4][:, bass.ds(512 * (si % 4), 512)]

                def q_mm(tb, hp):
                    """Project heads (2hp, 2hp+1) for t-block tb. Returns psum."""
                    qps = sps_pool.tile([128, 2, 512], f32, tag="sp",
                                        name=f"qps{tb}{hp}")
                    xqt = xq_tiles[tb]
                    for et in range(NE):
                        for i in range(2):
                            h = 2 * hp + i
                            nc.tensor.matmul(
                                qps[:, i], wq[:, et, bass.ts(h, 128)],
                                xqt[:, et, :], start=(et == 0),
                                stop=(et == NE - 1))
                    return qps

                def q_rope(tb, hp, qps):
                    for i in range(2):
                        rope(qT[2 * hp + i], qps[:, i], tb, rq_pool)

                def attn_qk(tb, h):
                    """Scores + exp for head h of t-block tb."""
                    nsi = 4 * (tb + 1)
                    eset = eG[h % 2]
                    for j in range(nsi // 2):
                        sp = sps_pool.tile([128, 2, 512], f32, tag="sp",
                                           name=f"sp{tb}{h}{j}")
                        for p2 in range(2):
                            si = 2 * j + p2
                            nc.tensor.matmul(
                                sp[:, p2], kT[h][:, bass.ts(si, 128)],
                                qT[h][:, bass.ts(tb, 512)], start=True,
                                stop=True)
                            v = si - 4 * tb
                            if v >= 0:
                                w = MW[v]
                                nc.vector.tensor_add(
                                    sp[:, p2, 0:w], sp[:, p2, 0:w],
                                    msk[:, bass.ds(MOFF[v], w)])
                        nc.scalar.activation(
                            eset[j // 2][:, bass.ts(j % 2, 1024)],
                            sp[:].rearrange("p a b -> p (a b)"), Exp)

                def attn_dv(tb, h):
                    """Denominator + A@V + normalization for head h."""
                    nsi = 4 * (tb + 1)
                    eset = eG[h % 2]
                    den = dps_pool.tile([128, 512], f32, tag="den",
                                        name=f"den{tb}{h}")
                    for si in range(nsi):
                        nc.tensor.matmul(den[:], ones[:], e_ap(eset, si),
                                         start=(si == 0), stop=(si == nsi - 1))
                    rec = rcp_pool.tile([128, 512], f32, tag="rec", name="rec")
                    nc.vector.reciprocal(rec[:], den[:])
                    up = ups_pool.tile([128, 512], f32, tag="up",
                                       name=f"up{tb}{h}")
                    for si in range(nsi):
                        g, sv = si // 4, si % 4
                        nc.tensor.matmul(up[:], vG[g][:, sv, bass.ts(h, 128)],
                                         e_ap(eset, si), start=(si == 0),
                                         stop=(si == nsi - 1))
                    nc.vector.tensor_mul(uT[h][:, bass.ts(tb, 512)], up[:],
                                         rec[:])

                def attn_block(tb, ropes=None):
                    """Full attention t-block with cross-head PE interleave."""
                    attn_qk(tb, 0)
                    attn_qk(tb, 1)
                    attn_dv(tb, 0)
                    if ropes is not None:
                        ropes()
                    attn_qk(tb, 2)
                    attn_dv(tb, 1)
                    attn_qk(tb, 3)
                    attn_dv(tb, 2)
                    attn_dv(tb, 3)

                qps = q_mm(0, 0)
                q_rope(0, 0, qps)
                qps = q_mm(0, 1)
                q_rope(0, 1, qps)
                for tb in range(1, NB):
                    if tb + 1 < NB:
                        load_xq(tb + 1)
                    qps0 = q_mm(tb, 0)
                    attn_block(tb - 1, ropes=lambda: q_rope(tb, 0, qps0))
                    qps1 = q_mm(tb, 1)
                    q_rope(tb, 1, qps1)
                attn_block(NB - 1)

            # ---------------- Phase 4: output projection ----------------
            with nc.named_scope("out_proj"), \
                 tc.tile_pool(name="ops", bufs=2, space="PSUM") as ops_pool, \
                 tc.tile_pool(name="ob", bufs=3) as ob_pool:
                for tt in range(NT):
                    op = ops_pool.tile([128, E], f32, tag="op", name=f"op{tt}")
                    for ec in range(4):
                        for h in range(HL):
                            nc.tensor.matmul(
                                op[:, bass.ts(ec, 512)],
                                uT[h][:, bass.ts(tt, 128)],
                                wo[:, h, bass.ts(ec, 512)],
                                start=(h == 0), stop=(h == HL - 1))
                    ob = ob_pool.tile([128, E], bf16, tag="ob", name=f"ob{tt}")
                    nc.scalar.copy(ob[:], op[:])
                    nc.sync.dma_start(out_d[tt], ob[:])

    nc.compile()
    return nc


def _get_module():
    if "nc" not in _NC_CACHE:
        _NC_CACHE["nc"] = _build_module()
    return _NC_CACHE["nc"]


def _host_prep(inputs_q, inputs_kv, positions, Wq, Wk, Wv, Wo):
    """Build the 8 per-core input maps (device-packed layouts, bf16)."""
    import ml_dtypes
    bf16 = ml_dtypes.bfloat16
    perm = np.concatenate([np.arange(0, D, 2), np.arange(1, D, 2)])  # de-interleave
    scale = np.float32(1.0 / np.sqrt(D))
    half = D // 2
    timescale = 10000.0 ** (2.0 * np.arange(half, dtype=np.float64) / D)
    ones = np.ones((128, 128), dtype=bf16)
    # mask variant v (diag sub-block at cols [128v, 128v+128)), width-trimmed:
    # masked (additive MASK_VALUE) where col < 128*v + row
    s_i = np.arange(128)[:, None]
    msk = np.concatenate(
        [np.where(np.arange(MW[v])[None, :] < 128 * v + s_i, MASK_VALUE, 0.0)
         for v in range(4)], axis=1).astype(np.float32)

    def pack_x(xT):
        # [E, S] f32 -> [NB, 128, NE, 512]: x_p[tb, p, et, t] = xT[128 et + p, 512 tb + t]
        return np.ascontiguousarray(
            xT.reshape(NE, 128, NB, 512).transpose(2, 1, 0, 3).astype(bf16))

    def pack_w(w):
        # [E, ND] -> [128, NE, ND]: w_p[p, et, n] = w[128 et + p, n]
        return np.ascontiguousarray(
            w.reshape(NE, 128, ND).transpose(1, 0, 2).astype(bf16))

    in_maps = []
    for c in range(8):
        b = c // 4
        h0 = (c % 4) * HL
        angle = positions[b].astype(np.float64)[None, :] / timescale[:, None]  # [64,S]
        cs = np.cos(angle).astype(np.float32)
        sn = np.sin(angle).astype(np.float32)
        csd = np.concatenate([cs, cs], axis=0)               # [128, S]
        sns = np.concatenate([-sn, sn], axis=0)              # [128, S]
        wq = (Wq[:, h0:h0 + HL, :][:, :, perm] * scale).reshape(E, ND)
        wk = Wk[:, h0:h0 + HL, :][:, :, perm].reshape(E, ND)
        wv = Wv[:, h0:h0 + HL, :].reshape(E, ND)
        wo = Wo[h0:h0 + HL]                                   # [HL, D, E]
        in_maps.append({
            "xq_p": pack_x(np.asarray(inputs_q[b]).T),
            "xkv_p": pack_x(np.asarray(inputs_kv[b]).T),
            "wq_p": pack_w(np.asarray(wq, dtype=np.float32)),
            "wk_p": pack_w(np.asarray(wk, dtype=np.float32)),
            "wv_p": pack_w(np.asarray(wv, dtype=np.float32)),
            "wo_p": np.ascontiguousarray(
                np.asarray(wo, dtype=np.float32).transpose(1, 0, 2).astype(bf16)),
            "csd": csd, "sns": sns, "ones": ones, "msk": msk,
        })
    return in_maps


def kernel(inputs_q, inputs_kv, positions, Wq, Wk, Wv, Wo, _trace=False,
           _trace_kwargs=None):
    from concourse import bass_utils

    nc = _get_module()
    in_maps = _host_prep(inputs_q, inputs_kv, positions, Wq, Wk, Wv, Wo)
    res = bass_utils.run_bass_kernel_spmd(
        nc, in_maps, core_ids=list(range(8)), trace=_trace,
        **(_trace_kwargs or {}))
    if _trace:
        _NC_CACHE["last_results"] = res
    parts = [np.asarray(res.results[c]["out"], dtype=np.float32).reshape(S, E)
             for c in range(8)]
    out0 = parts[0] + parts[1] + parts[2] + parts[3]
    out1 = parts[4] + parts[5] + parts[6] + parts[7]
    return np.stack([out0, out1]).astype(np.float32)
